# revision 35
# baseline (speedup 1.0000x reference)
"""Trainium2 Bass kernel for nn_CapsuleLayer (dynamic routing capsule layer).

Reference computation (per batch element b):
    u_hat[b,r,c,o] = sum_i W[r,c,o,i] * x[b,r,i]        (R=1152, C=10, O=16, I=8)
    b_ij = 0
    3 routing iterations:
        c_ij = softmax(b_ij, axis=r)
        s_j[c,o] = sum_r c_ij[r,c] * u_hat[r,c,o]
        v = squash(s_j)  over o
        b_ij += sum_o u_hat[r,c,o] * v[c,o]   (except last iteration)
    output v -> [B, 1, C, O, 1]

Sharding: data-parallel over batch B=256 across 8 cores (32 each), W replicated.

Per-core layouts (host numpy prepacks everything; engine SBUF access patterns
must be partition-contiguous and start at partition 0/32/64/96, so every
on-chip tensor here is addressed at partition base 0):
  K-partition index p = 8*rh + i over (rh in [0,16), i in [0,8)); r = 16*ch+rh,
  ch in [0,72).  Column order J = c*32 + b is shared by b_ij, exp, y, the
  s-diagonal PSUM, the squash pipeline and the Vm matrices.

  - u_hat built on PE via block-diagonal "D" matmuls (f16 for precision):
      psum[co, (b, rh')] = sum_p Wp16[p, ch, co] * Df[ch, p, b*16+rh']
    Df columns are (b, rh') so the PSUM->SBUF drains are 16-contiguous:
    U1a[16c+o, b, r] (c<8) via DVE, U1b4[32*bq + 16*cc + o, bg, r]
    (c = 8+cc, b = 8*bq+bg) via the scalar (ACT) engine - both engines
    run the drains in parallel under the PE matmuls.
  - iteration-0 s_j is fused into phase 1: per ch two compact matmuls
    accumulate s0[(c,o), b] (c<8 full 128 rows + 32-row tail) from
    Wp16 x xp16; after the loop an eye-weighted reshape matmul scatters
    s0 into the [o=16, (c,b)=320] squash layout (replaces the old
    separate 720-matmul iteration-0 pass).
  - b_ij update, r-major directly: per (b, r-chunk-128): lhsT = U1a[:, b, rch]
    (f16 FWL weight loads), rhs = 8-col slice of block-diag(v) Vm1; the tail
    batches 4 b's per matmul via U1b4/Vm24.  All outputs land in one
    [128, 384] J-layout PSUM tile per r-chunk; one DVE add accumulates it
    into b_sb[128, rc, J].
  - softmax (no max subtraction: logits stay small): one ACT Exp pass ->
    erm bf16; Z via ones16-matmul over r partitions (9 accumulating matmuls
    into a [16, 320] PSUM row-block); reciprocal -> rz16 [16, 320].
  - s_j diagonal: per ch: rep = EM64-expand matmul (i-replication of 16 exp
    rows out of a 64-row aligned block); y = rep * xp broadcast over c; then
    per c: psum[0:16, 32c:32c+32] += Wpb[:, ch, 16c:16c+16]^T @ y[:, 32c:+32]
    -- the c-diagonal lands as column blocks of a single [16, 320] region.
  - squash entirely in the [o=16, (c, b)] layout: s~ = s * rz; sum_o via a
    [16,1]-ones matmul -> [1, 320]; scale chain on [1, 320]; scale
    replicated back to 16 rows by a K=1 ones matmul; v = s~ * scale.
    Dummy 1-element ACT ops prefetch the Exp/Sqrt tables during engine-idle
    windows so the table swaps stay off the critical path.
  - Vm fill without partition gymnastics: vFull = EXP-matmul replicating vD's
    o-rows to all 128 capsule rows, then Vm = vFull * MASK (host constants
    carry the c-diagonal / batch-quad block structure).
  - output vD [o, (c, b)] fp32 -> DRAM [16, 320]; numpy -> [B,1,C,O,1].
"""

import sys

if "/opt/trn_rl_repo" not in sys.path:
    sys.path.insert(0, "/opt/trn_rl_repo")

import numpy as np
import ml_dtypes

import concourse.bass as bass
import concourse.mybir as mybir
from concourse import bacc
from concourse.tile import TileContext

BF16 = mybir.dt.bfloat16
F16 = mybir.dt.float16
F32 = mybir.dt.float32
NPBF16 = ml_dtypes.bfloat16

B, R, C, O, I = 256, 1152, 10, 16, 8
NCORES = 8
BC = B // NCORES          # 32 batch elements per core
CH = R // 16              # 72 chunks of 16 r's
EPS = 1e-7
ExpF = mybir.ActivationFunctionType.Exp
SqrtF = mybir.ActivationFunctionType.Sqrt


def _host_prep(xs, W):
    """Per-core input arrays. xs: [32,1152,8] f32, W: [1152,10,16,8] f32."""
    Wr = (
        W.reshape(CH, 16, C, O, I)
        .transpose(1, 4, 0, 2, 3)  # rh, i, ch, c, o
        .reshape(128, CH * 160)
    )
    Wp16 = Wr.astype(np.float16)
    Wpb = Wr.astype(NPBF16)
    xr = (
        xs.reshape(BC, CH, 16, I)
        .transpose(2, 3, 1, 0)  # rh, i, ch, b
        .reshape(128, CH * BC)
    )
    xp16 = xr.astype(np.float16)
    tmp = xs.reshape(BC, CH, 16, I).transpose(1, 2, 3, 0)  # ch, rh, i, b
    # D block-diag with (b, rh') column order -> 16-contiguous drains
    D6 = np.zeros((CH, 16, I, BC, 16), np.float32)
    for rh in range(16):
        D6[:, rh, :, :, rh] = tmp[:, rh, :, :]
    Df = D6.reshape(CH, 128, BC * 16).astype(np.float16)
    return {"Wp16": Wp16, "Wpb": Wpb, "xp16": xp16, "Df": Df}


def _host_consts():
    p = np.arange(128)
    # EM64[64k + 16*par + j2, par*128 + q] = (q//8 == j2): K=64 expand blocks
    # at partition bases 0/64; col group `par` selects which 16-row quarter of
    # the 64-block is replicated into the (rh, i) partition grid.
    EM64 = np.zeros((128, 4, 128), np.float32)
    for k in range(2):
        for par in range(4):
            for j2 in range(16):
                EM64[64 * k + 16 * par + j2, par, :] = (p // 8 == j2)
    EM64 = EM64.reshape(128, 512).astype(NPBF16)
    # EXP16[o', 16c+o] = (o'==o): replicate vD rows to all capsule rows
    EXP16 = np.tile(np.eye(16, dtype=np.float32), (1, 8)).astype(np.float16)
    # EXP24[o', 32bq+16cc+o] = (o'==o)
    EXP24 = np.tile(np.tile(np.eye(16, dtype=np.float32), (1, 2)),
                    (1, 4)).astype(np.float16)
    # MASK1[16c+o, c'*32+b] = (c==c') for c' < 8
    c_of = (p // 16)[:, None]
    cols = np.arange(256)[None, :] // 32
    MASK1 = (c_of == cols).astype(np.float16)
    # MASK24[32bq+16cc+o, bg*8+bq'*2+cc'] = (bq==bq') & (cc==cc')
    bq_of = (p // 32)[:, None]
    cc_of = ((p % 32) // 16)[:, None]
    col24 = np.arange(64)[None, :]
    MASK24 = ((bq_of == (col24 % 8) // 2) & (cc_of == col24 % 2)).astype(
        np.float16)
    EYE32F = np.eye(32, dtype=np.float32)
    return {"EM64": EM64, "EXP16": EXP16, "EXP24": EXP24, "MASK1": MASK1,
            "MASK24": MASK24, "EYE32F": EYE32F}


def build_nc(stop_after=2):
    nc = bacc.Bacc("TRN2", target_bir_lowering=False, debug=False,
                   num_devices=NCORES)
    dr = {}
    for name, shape, dt in [
        ("Wp16", [128, CH * 160], F16), ("Wpb", [128, CH * 160], BF16),
        ("xp16", [128, CH * BC], F16),
        ("Df", [CH, 128, 16 * BC], F16), ("EM64", [128, 512], BF16),
        ("EXP16", [16, 128], F16), ("EXP24", [16, 128], F16),
        ("MASK1", [128, 256], F16), ("MASK24", [128, 64], F16),
        ("EYE32F", [32, 32], F32),
    ]:
        dr[name] = nc.dram_tensor(name, shape, dt, kind="ExternalInput").ap()
    d_out = nc.dram_tensor("out", [O, 320], F32, kind="ExternalOutput").ap()

    with TileContext(nc) as tc:
        _emit(nc, tc, dr, d_out, stop_after)
    nc.compile()
    return nc


def _emit(nc, tc, dr, d_out, stop_after=2):
    from contextlib import ExitStack

    with ExitStack() as ctx:
        consts = ctx.enter_context(tc.tile_pool(name="consts", bufs=1))
        upool = ctx.enter_context(tc.tile_pool(name="upool", bufs=1))
        bpool = ctx.enter_context(tc.tile_pool(name="bpool", bufs=1))
        dstream = ctx.enter_context(tc.tile_pool(name="dstream", bufs=8))
        ystream = ctx.enter_context(tc.tile_pool(name="ystream", bufs=3))

        # ---- resident tiles ----
        Wp16 = consts.tile([128, CH, 160], F16)
        Wpb = consts.tile([128, CH, 160], BF16)
        xp16 = consts.tile([128, CH, BC], F16)
        EM64 = consts.tile([128, 4, 128], BF16)
        EXP16 = consts.tile([16, 128], F16)
        EXP24 = consts.tile([16, 128], F16)
        MASK1 = consts.tile([128, 256], F16)
        MASK24 = consts.tile([128, 64], F16)
        EYE32F = consts.tile([32, 32], F32)
        ones16 = consts.tile([128, 16], BF16)
        onesf = consts.tile([16, 1], F16)
        ones1 = consts.tile([1, 16], F16)
        epsb = consts.tile([128, 1], F32)
        scratch1 = consts.tile([1, 1], F32)

        # critical-path inputs first, in 9-chunk slices interleaved so the
        # first phase-1 matmuls can start as soon as slice 0 lands
        xp16f = xp16[:].rearrange("p a b -> p (a b)")
        Wp16f = Wp16[:].rearrange("p a b -> p (a b)")
        Wpbf = Wpb[:].rearrange("p a b -> p (a b)")
        for k in range(8):
            slx = slice(k * 9 * BC, (k + 1) * 9 * BC)
            nc.sync.dma_start(out=xp16f[:, slx], in_=dr["xp16"][:, slx])
            sl = slice(k * 9 * 160, (k + 1) * 9 * 160)
            nc.sync.dma_start(out=Wp16f[:, sl], in_=dr["Wp16"][:, sl])
        nc.gpsimd.memset(ones16[:], 1.0)
        nc.gpsimd.memset(onesf[:], 1.0)
        nc.gpsimd.memset(ones1[:], 1.0)
        nc.gpsimd.memset(epsb[:], EPS)

        U1a = upool.tile([128, BC, R], F16)     # [16c+o (c<8), b, r]
        U1b4 = upool.tile([128, 8, R], F16)     # [32bq+16cc+o, bg, r]

        bsb = bpool.tile([128, 9, 320], F32)    # b_ij r-major, cols J=c*32+b
        erm = bpool.tile([128, 9, 320], BF16)   # exp(b_ij)
        rz16 = bpool.tile([16, 320], F32)       # 1/Z on 16 o-rows
        Vm1 = bpool.tile([128, 256], F16)       # block-diag v, cols c*32+b
        Vm24 = bpool.tile([128, 64], F16)       # [32bq+16cc+o, (bq',cc',bg)]
        s0sb = bpool.tile([32, 160], F32)       # fused it0 s_j, [b, (c,o)]

        # preload the Sqrt table while phase 1 runs (squash(0) needs it)
        nc.scalar.activation(scratch1[:], epsb[0:1, :], SqrtF)

        # ===== Phase 1: build u_hat + fused iteration-0 s_j =====
        with tc.tile_pool(name="ppbuild", bufs=4, space="PSUM") as ppb1, \
             tc.tile_pool(name="ppbuild2", bufs=3, space="PSUM") as ppb2, \
             tc.tile_pool(name="pps0", bufs=1, space="PSUM") as pps0:
            s0p = pps0.tile([32, 160], F32)
            for ch in range(CH):
                dfc = dstream.tile([128, 16 * BC], F16, tag="dfc")
                nc.sync.dma_start(out=dfc[:], in_=dr["Df"][ch])
                pa = ppb1.tile([128, 16 * BC], F32, tag="pa")
                pb = ppb2.tile([32, 16 * BC], F32, tag="pb")
                nc.tensor.matmul(pa[:], Wp16[:, ch, 0:128], dfc[:],
                                 start=True, stop=True)
                nc.tensor.matmul(pb[:], Wp16[:, ch, 128:160], dfc[:],
                                 start=True, stop=True)
                if 16 <= ch < 20:
                    k = ch - 16
                    sl = slice(k * 18 * 160, (k + 1) * 18 * 160)
                    nc.sync.dma_start(out=Wpbf[:, sl], in_=dr["Wpb"][:, sl])
                elif ch == 20:
                    nc.sync.dma_start(
                        out=EM64[:].rearrange("p a b -> p (a b)"),
                        in_=dr["EM64"])
                elif ch == 21:
                    for nm, t in [("EXP16", EXP16), ("EXP24", EXP24),
                                  ("MASK1", MASK1), ("MASK24", MASK24),
                                  ("EYE32F", EYE32F)]:
                        nc.sync.dma_start(out=t[:], in_=dr[nm])
                # fused iteration-0 s_j, transposed [b, (c,o)]: xp as the
                # (cheap, 32-row) stationary side, one matmul per chunk
                nc.tensor.matmul(s0p[:], xp16[:, ch, :], Wp16[:, ch, :],
                                 start=(ch == 0), stop=(ch == CH - 1),
                                 skip_group_check=True)
                # drains (psum cols (b, rh') -> 16-contiguous dst runs),
                # alternating whole U1a copies between DVE and ACT and
                # splitting the batch-quad tail copies 5:3 per chunk pair
                pa_v = pa[:].rearrange("p (b h) -> p b h", b=BC)
                ua_dst = U1a[:, :, 16 * ch:16 * ch + 16]
                if ch % 2 == 0:
                    nc.vector.tensor_copy(ua_dst, pa_v)
                else:
                    nc.scalar.copy(ua_dst, pa_v)
                pb_v = pb[:].rearrange("p (b h) -> p b h", b=BC)
                for bq in range(4):
                    dst = U1b4[32 * bq:32 * bq + 32, :, 16 * ch:16 * ch + 16]
                    src = pb_v[:, 8 * bq:8 * bq + 8, :]
                    on_dve = bq < 3 if ch % 2 == 0 else bq < 2
                    if on_dve:
                        nc.vector.tensor_copy(dst, src)
                    else:
                        nc.scalar.copy(dst, src)
            nc.vector.tensor_copy(s0sb[:], s0p[:])

        # routing-phase PSUM pools (opened after the build pools close)
        pp_bb = ctx.enter_context(tc.tile_pool(name="ppbb", bufs=2,
                                               space="PSUM"))
        pp_z = ctx.enter_context(tc.tile_pool(name="ppz", bufs=1, space="PSUM"))
        pp_s = ctx.enter_context(tc.tile_pool(name="pps", bufs=1, space="PSUM"))
        pp_s2 = ctx.enter_context(tc.tile_pool(name="pps2", bufs=1,
                                               space="PSUM"))
        pp_rep = ctx.enter_context(
            tc.tile_pool(name="pprep", bufs=3, space="PSUM"))
        pps = pp_s.tile([128, 512], F32)    # s-diag [16, 0:320] | sq [1,320:352]
        pps2 = pp_s2.tile([128, 512], F32)  # srep [16, 0:320]

        # transpose fused s0 [b, (c,o)] -> pps [16 o, (c, b)]: per c one
        # matmul with s0sb as stationary side against a 32x32 identity
        for c in range(C):
            nc.tensor.matmul(pps[0:16, 32 * c:32 * c + 32],
                             s0sb[:, 16 * c:16 * c + 16], EYE32F[:],
                             start=(c == 0), stop=(c == C - 1),
                             skip_group_check=True)

        def squash(it):
            """pps[0:16, 0:320] holds s-diagonal; emits v + Vm fills / output."""
            stf = bpool.tile([16, 320], F32, tag="stf")
            if it == 0:
                nc.scalar.mul(stf[:], pps[0:16, 0:320], 1.0 / R)
            else:
                nc.vector.tensor_mul(stf[:], pps[0:16, 0:320], rz16[:])
            sq2 = bpool.tile([16, 320], F16, tag="sq2")
            nc.vector.tensor_mul(sq2[:], stf[:], stf[:])
            # sum over o: [16,1]-ones matmul -> [1, 320] at partition base 32
            nc.tensor.matmul(pps2[32:33, 0:320], onesf[:], sq2[:],
                             start=True, stop=True)
            # short chain, all reading sq straight from PSUM:
            #   rt = sqrt(sq+eps); den = (sq+1)*rt; scl = sq/den
            sqp = pps2[32:33, 0:320]
            rt = bpool.tile([1, 320], F32, tag="rt")
            den = bpool.tile([1, 320], F32, tag="den")
            inv = bpool.tile([1, 320], F32, tag="inv")
            scl = bpool.tile([1, 320], F16, tag="scl")
            nc.scalar.activation(rt[:], sqp, SqrtF, bias=epsb[0:1, :])
            if it < 2:
                # prefetch the Exp table for the next iteration's softmax
                nc.scalar.activation(scratch1[:], epsb[0:1, :], ExpF)
            nc.vector.scalar_tensor_tensor(
                den[:], sqp, 1.0, rt[:],
                mybir.AluOpType.add, mybir.AluOpType.mult)
            nc.vector.reciprocal_approx_fast(inv[:], den[:])
            nc.vector.tensor_mul(scl[:], sqp, inv[:])
            nc.tensor.matmul(pps2[0:16, 0:320], ones1[:], scl[:],
                             start=True, stop=True)
            if it < 2:
                vD = bpool.tile([16, 320], F16, tag="vD")
                nc.vector.tensor_mul(vD[:], stf[:], pps2[0:16, 0:320])
                ppv = pp_rep.tile([128, 320], F32, tag="prep")
                nc.tensor.matmul(ppv[0:128, 0:256], EXP16[:], vD[:, 0:256],
                                 start=True, stop=True)
                v24 = vD[:, 256:320].rearrange(
                    "p (c q g) -> p g q c", c=2, g=8)
                nc.tensor.matmul(ppv[0:128, 256:320], EXP24[:], v24,
                                 start=True, stop=True)
                nc.vector.tensor_mul(Vm1[:], ppv[0:128, 0:256], MASK1[:])
                nc.vector.tensor_mul(Vm24[:], ppv[0:128, 256:320], MASK24[:])
            else:
                vD = bpool.tile([16, 320], F32, tag="vDf")
                nc.vector.tensor_mul(vD[:], stf[:], pps2[0:16, 0:320])
                nc.sync.dma_start(out=d_out[:], in_=vD[:])

        squash(0)

        # ================= routing iterations =================
        Vm1v = Vm1[:].rearrange("p (c b) -> p c b", b=BC)       # [128,8,32]
        for it in (1, 2):
            if it > stop_after:
                break
            # software pipeline: b_ij-update block rc on the PE is followed
            # by the rep/y/s_j work of block rc-1, so the PE never drains
            # while DVE (badd/y-mult) and ACT (exp) run under it.
            pz = pp_z.tile([16, 320], F32, tag="pz")

            def bupd_block(rc):
                r0 = 128 * rc
                pbb = pp_bb.tile([128, 384], F32, tag="pbb")
                pbv = pbb[:, 0:320].rearrange("p (c b) -> p c b", b=BC)
                for b in range(BC):
                    nc.tensor.matmul(pbv[:, 0:8, b],
                                     U1a[:, b, r0:r0 + 128],
                                     Vm1v[:, :, b],
                                     start=True, stop=True)
                for bg in range(8):
                    nc.tensor.matmul(
                        pbb[:, 320 + 8 * bg:328 + 8 * bg],
                        U1b4[:, bg, r0:r0 + 128],
                        Vm24[:, 8 * bg:8 * bg + 8],
                        start=True, stop=True)
                # tail cols (bg, bq', cc') -> J-cols 256 + cc'*32 + 8bq' + bg
                h2src = pbb[:, 320:384].rearrange(
                    "p (g q c) -> p g q c", q=4, c=2)
                h2dst = bsb[:, rc, 256:320].rearrange(
                    "p (c q g) -> p g q c", q=4, g=8)
                if it == 1:
                    nc.vector.tensor_copy(bsb[:, rc, 0:256], pbb[:, 0:256])
                    nc.vector.tensor_copy(h2dst, h2src)
                    # exp straight from PSUM, in parallel with the bsb copy
                    nc.scalar.activation(erm[:, rc, 0:256], pbb[:, 0:256],
                                         ExpF)
                    e2dst = erm[:, rc, 256:320].rearrange(
                        "p (c q g) -> p g q c", q=4, g=8)
                    nc.scalar.activation(e2dst, h2src, ExpF)
                else:
                    nc.vector.tensor_add(bsb[:, rc, 0:256], bsb[:, rc, 0:256],
                                         pbb[:, 0:256])
                    nc.vector.tensor_add(h2dst, h2dst, h2src)
                    nc.scalar.activation(erm[:, rc, :], bsb[:, rc, :], ExpF)

            def sj_block(rc):
                nc.tensor.matmul(pz[:], ones16[:], erm[:, rc, :],
                                 start=(rc == 0), stop=(rc == 8),
                                 skip_group_check=True)
                for q in range(8):
                    ch = 8 * rc + q
                    m, par = q // 4, q % 4
                    prep = pp_rep.tile([128, 320], F32, tag="prep")
                    nc.tensor.matmul(prep[:],
                                     EM64[64 * m:64 * m + 64, par, :],
                                     erm[64 * m:64 * m + 64, rc, :],
                                     start=True, stop=True)
                    yc = ystream.tile([128, 320], BF16, tag="yc")
                    xb = xp16[:, ch:ch + 1, :].broadcast_to([128, 10, BC])
                    nc.vector.tensor_mul(
                        yc[:].rearrange("p (c b) -> p c b", b=BC),
                        prep[:].rearrange("p (c b) -> p c b", b=BC), xb)
                    for c in range(C):
                        nc.tensor.matmul(pps[0:16, 32 * c:32 * c + 32],
                                         Wpb[:, ch, 16 * c:16 * c + 16],
                                         yc[:, 32 * c:32 * c + 32],
                                         start=(ch == 0 and c == 0),
                                         stop=(ch == CH - 1 and c == C - 1),
                                         skip_group_check=True)

            for rc in range(9):
                bupd_block(rc)
                if rc >= 1:
                    sj_block(rc - 1)
            # prefetch the Sqrt table for this iteration's squash
            nc.scalar.activation(scratch1[:], epsb[0:1, :], SqrtF)
            sj_block(8)
            nc.vector.reciprocal_approx_fast(rz16[:], pz[:])
            squash(it)


_NC_CACHE = None


def _get_nc():
    global _NC_CACHE
    if _NC_CACHE is None:
        _NC_CACHE = build_nc()
    return _NC_CACHE


def decode_out(o):
    """[16, 320] core output (o, (c, b)) -> [32, 10, 16] (b, c, o)."""
    return o.reshape(O, C, BC).transpose(2, 1, 0)


def kernel(x, W):
    """Full-input entry point. x: [256,1152,8] f32, W: [1152,10,16,8] f32."""
    from concourse.bass_utils import run_bass_kernel_spmd

    x = np.asarray(x, np.float32)
    W = np.asarray(W, np.float32)
    nc = _get_nc()
    consts = _host_consts()
    in_maps = []
    for k in range(NCORES):
        m = _host_prep(x[k * BC:(k + 1) * BC], W)
        m.update(consts)
        in_maps.append(m)
    res = run_bass_kernel_spmd(nc, in_maps, core_ids=list(range(NCORES)))
    v = np.concatenate([decode_out(res.results[k]["out"])
                        for k in range(NCORES)], axis=0)  # [256, 10, 16]
    return v[:, None, :, :, None].astype(np.float32)


# revision 36
# speedup vs baseline: 1.0181x; 1.0181x over previous
"""Trainium2 Bass kernel for nn_CapsuleLayer (dynamic routing capsule layer).

Reference computation (per batch element b):
    u_hat[b,r,c,o] = sum_i W[r,c,o,i] * x[b,r,i]        (R=1152, C=10, O=16, I=8)
    b_ij = 0
    3 routing iterations:
        c_ij = softmax(b_ij, axis=r)
        s_j[c,o] = sum_r c_ij[r,c] * u_hat[r,c,o]
        v = squash(s_j)  over o
        b_ij += sum_o u_hat[r,c,o] * v[c,o]   (except last iteration)
    output v -> [B, 1, C, O, 1]

Sharding: data-parallel over batch B=256 across 8 cores (32 each), W replicated.

Per-core layouts (host numpy prepacks everything; engine SBUF access patterns
must be partition-contiguous and start at partition 0/32/64/96, so every
on-chip tensor here is addressed at partition base 0):
  K-partition index p = 8*rh + i over (rh in [0,16), i in [0,8)); r = 16*ch+rh,
  ch in [0,72).  Column order J = c*32 + b is shared by b_ij, exp, y, the
  s-diagonal PSUM, the squash pipeline and the Vm matrices.

  - u_hat built on PE via block-diagonal "D" matmuls (f16 for precision):
      psum[co, (b, rh')] = sum_p Wp16[p, ch, co] * Df[ch, p, b*16+rh']
    Df columns are (b, rh') so the PSUM->SBUF drains are 16-contiguous:
    U1a[16c+o, b, r] (c<8) via DVE, U1b4[32*bq + 16*cc + o, bg, r]
    (c = 8+cc, b = 8*bq+bg) via the scalar (ACT) engine - both engines
    run the drains in parallel under the PE matmuls.
  - iteration-0 s_j is fused into phase 1: per ch two compact matmuls
    accumulate s0[(c,o), b] (c<8 full 128 rows + 32-row tail) from
    Wp16 x xp16; after the loop an eye-weighted reshape matmul scatters
    s0 into the [o=16, (c,b)=320] squash layout (replaces the old
    separate 720-matmul iteration-0 pass).
  - b_ij update, r-major directly: per (b, r-chunk-128): lhsT = U1a[:, b, rch]
    (f16 FWL weight loads), rhs = 8-col slice of block-diag(v) Vm1; the tail
    batches 4 b's per matmul via U1b4/Vm24.  All outputs land in one
    [128, 384] J-layout PSUM tile per r-chunk; one DVE add accumulates it
    into b_sb[128, rc, J].
  - softmax (no max subtraction: logits stay small): one ACT Exp pass ->
    erm bf16; Z via ones16-matmul over r partitions (9 accumulating matmuls
    into a [16, 320] PSUM row-block); reciprocal -> rz16 [16, 320].
  - s_j diagonal: per ch: rep = EM64-expand matmul (i-replication of 16 exp
    rows out of a 64-row aligned block); y = rep * xp broadcast over c; then
    per c: psum[0:16, 32c:32c+32] += Wpb[:, ch, 16c:16c+16]^T @ y[:, 32c:+32]
    -- the c-diagonal lands as column blocks of a single [16, 320] region.
  - squash entirely in the [o=16, (c, b)] layout: s~ = s * rz; sum_o via a
    [16,1]-ones matmul -> [1, 320]; scale chain on [1, 320]; scale
    replicated back to 16 rows by a K=1 ones matmul; v = s~ * scale.
    Dummy 1-element ACT ops prefetch the Exp/Sqrt tables during engine-idle
    windows so the table swaps stay off the critical path.
  - Vm fill without partition gymnastics: vFull = EXP-matmul replicating vD's
    o-rows to all 128 capsule rows, then Vm = vFull * MASK (host constants
    carry the c-diagonal / batch-quad block structure).
  - output vD [o, (c, b)] fp32 -> DRAM [16, 320]; numpy -> [B,1,C,O,1].
"""

import sys

if "/opt/trn_rl_repo" not in sys.path:
    sys.path.insert(0, "/opt/trn_rl_repo")

import numpy as np
import ml_dtypes

import concourse.bass as bass
import concourse.mybir as mybir
from concourse import bacc
from concourse.tile import TileContext

BF16 = mybir.dt.bfloat16
F16 = mybir.dt.float16
F32 = mybir.dt.float32
NPBF16 = ml_dtypes.bfloat16

B, R, C, O, I = 256, 1152, 10, 16, 8
NCORES = 8
BC = B // NCORES          # 32 batch elements per core
CH = R // 16              # 72 chunks of 16 r's
EPS = 1e-7
ExpF = mybir.ActivationFunctionType.Exp
SqrtF = mybir.ActivationFunctionType.Sqrt


def _host_prep(xs, W):
    """Per-core input arrays. xs: [32,1152,8] f32, W: [1152,10,16,8] f32."""
    Wr = (
        W.reshape(CH, 16, C, O, I)
        .transpose(1, 4, 0, 2, 3)  # rh, i, ch, c, o
        .reshape(128, CH * 160)
    )
    Wp16 = Wr.astype(np.float16)
    Wpb = Wr.astype(NPBF16)
    xr = (
        xs.reshape(BC, CH, 16, I)
        .transpose(2, 3, 1, 0)  # rh, i, ch, b
        .reshape(128, CH * BC)
    )
    xp16 = xr.astype(np.float16)
    tmp = xs.reshape(BC, CH, 16, I).transpose(1, 2, 3, 0)  # ch, rh, i, b
    # D block-diag with (b, rh') column order -> 16-contiguous drains
    D6 = np.zeros((CH, 16, I, BC, 16), np.float32)
    for rh in range(16):
        D6[:, rh, :, :, rh] = tmp[:, rh, :, :]
    Df = D6.reshape(CH, 128, BC * 16).astype(np.float16)
    return {"Wp16": Wp16, "Wpb": Wpb, "xp16": xp16, "Df": Df}


def _host_consts():
    p = np.arange(128)
    # EM64[64k + 16*par + j2, par*128 + q] = (q//8 == j2): K=64 expand blocks
    # at partition bases 0/64; col group `par` selects which 16-row quarter of
    # the 64-block is replicated into the (rh, i) partition grid.
    EM64 = np.zeros((128, 4, 128), np.float32)
    for k in range(2):
        for par in range(4):
            for j2 in range(16):
                EM64[64 * k + 16 * par + j2, par, :] = (p // 8 == j2)
    EM64 = EM64.reshape(128, 512).astype(NPBF16)
    # EXP16[o', 16c+o] = (o'==o): replicate vD rows to all capsule rows
    EXP16 = np.tile(np.eye(16, dtype=np.float32), (1, 8)).astype(np.float16)
    # EXP24[o', 32bq+16cc+o] = (o'==o)
    EXP24 = np.tile(np.tile(np.eye(16, dtype=np.float32), (1, 2)),
                    (1, 4)).astype(np.float16)
    # MASK1[16c+o, c'*32+b] = (c==c') for c' < 8
    c_of = (p // 16)[:, None]
    cols = np.arange(256)[None, :] // 32
    MASK1 = (c_of == cols).astype(np.float16)
    # MASK24[32bq+16cc+o, bg*8+bq'*2+cc'] = (bq==bq') & (cc==cc')
    bq_of = (p // 32)[:, None]
    cc_of = ((p % 32) // 16)[:, None]
    col24 = np.arange(64)[None, :]
    MASK24 = ((bq_of == (col24 % 8) // 2) & (cc_of == col24 % 2)).astype(
        np.float16)
    EYE32F = np.eye(32, dtype=np.float32)
    return {"EM64": EM64, "EXP16": EXP16, "EXP24": EXP24, "MASK1": MASK1,
            "MASK24": MASK24, "EYE32F": EYE32F}


def build_nc(stop_after=2):
    nc = bacc.Bacc("TRN2", target_bir_lowering=False, debug=False,
                   num_devices=NCORES)
    dr = {}
    for name, shape, dt in [
        ("Wp16", [128, CH * 160], F16), ("Wpb", [128, CH * 160], BF16),
        ("xp16", [128, CH * BC], F16),
        ("Df", [CH, 128, 16 * BC], F16), ("EM64", [128, 512], BF16),
        ("EXP16", [16, 128], F16), ("EXP24", [16, 128], F16),
        ("MASK1", [128, 256], F16), ("MASK24", [128, 64], F16),
        ("EYE32F", [32, 32], F32),
    ]:
        dr[name] = nc.dram_tensor(name, shape, dt, kind="ExternalInput").ap()
    d_out = nc.dram_tensor("out", [O, 320], F32, kind="ExternalOutput").ap()

    with TileContext(nc) as tc:
        _emit(nc, tc, dr, d_out, stop_after)
    nc.compile()
    return nc


def _emit(nc, tc, dr, d_out, stop_after=2):
    from contextlib import ExitStack

    with ExitStack() as ctx:
        consts = ctx.enter_context(tc.tile_pool(name="consts", bufs=1))
        upool = ctx.enter_context(tc.tile_pool(name="upool", bufs=1))
        bpool = ctx.enter_context(tc.tile_pool(name="bpool", bufs=1))
        dstream = ctx.enter_context(tc.tile_pool(name="dstream", bufs=8))
        ystream = ctx.enter_context(tc.tile_pool(name="ystream", bufs=3))

        # ---- resident tiles ----
        Wp16 = consts.tile([128, CH, 160], F16)
        Wpb = consts.tile([128, CH, 160], BF16)
        xp16 = consts.tile([128, CH, BC], F16)
        EM64 = consts.tile([128, 4, 128], BF16)
        EXP16 = consts.tile([16, 128], F16)
        EXP24 = consts.tile([16, 128], F16)
        MASK1 = consts.tile([128, 256], F16)
        MASK24 = consts.tile([128, 64], F16)
        EYE32F = consts.tile([32, 32], F32)
        ones16 = consts.tile([128, 16], BF16)
        onesf = consts.tile([16, 1], F16)
        ones1 = consts.tile([1, 16], F16)
        epsb = consts.tile([128, 1], F32)
        scratch1 = consts.tile([1, 1], F32)

        # critical-path inputs first, in 9-chunk slices interleaved so the
        # first phase-1 matmuls can start as soon as slice 0 lands
        xp16f = xp16[:].rearrange("p a b -> p (a b)")
        Wp16f = Wp16[:].rearrange("p a b -> p (a b)")
        Wpbf = Wpb[:].rearrange("p a b -> p (a b)")
        for k in range(8):
            slx = slice(k * 9 * BC, (k + 1) * 9 * BC)
            nc.sync.dma_start(out=xp16f[:, slx], in_=dr["xp16"][:, slx])
            sl = slice(k * 9 * 160, (k + 1) * 9 * 160)
            nc.sync.dma_start(out=Wp16f[:, sl], in_=dr["Wp16"][:, sl])
        nc.gpsimd.memset(ones16[:], 1.0)
        nc.gpsimd.memset(onesf[:], 1.0)
        nc.gpsimd.memset(ones1[:], 1.0)
        nc.gpsimd.memset(epsb[:], EPS)

        U1a = upool.tile([128, BC, R], F16)     # [16c+o (c<8), b, r]
        U1b4 = upool.tile([128, 8, R], F16)     # [32bq+16cc+o, bg, r]

        bsb = bpool.tile([128, 9, 320], F32)    # b_ij r-major, cols J=c*32+b
        erm = bpool.tile([128, 9, 320], BF16)   # exp(b_ij)
        rz16 = bpool.tile([16, 320], F32)       # 1/Z on 16 o-rows
        Vm1 = bpool.tile([128, 256], F16)       # block-diag v, cols c*32+b
        Vm24 = bpool.tile([128, 64], F16)       # [32bq+16cc+o, (bq',cc',bg)]
        s0sb = bpool.tile([32, 160], F32)       # fused it0 s_j, [b, (c,o)]

        # preload the Sqrt table while phase 1 runs (squash(0) needs it)
        nc.scalar.activation(scratch1[:], epsb[0:1, :], SqrtF)

        # ===== Phase 1: build u_hat + fused iteration-0 s_j =====
        with tc.tile_pool(name="ppbuild", bufs=4, space="PSUM") as ppb1, \
             tc.tile_pool(name="ppbuild2", bufs=3, space="PSUM") as ppb2, \
             tc.tile_pool(name="pps0", bufs=1, space="PSUM") as pps0:
            s0p = pps0.tile([32, 160], F32)
            for ch in range(CH):
                dfc = dstream.tile([128, 16 * BC], F16, tag="dfc")
                nc.sync.dma_start(out=dfc[:], in_=dr["Df"][ch])
                pa = ppb1.tile([128, 16 * BC], F32, tag="pa")
                pb = ppb2.tile([32, 16 * BC], F32, tag="pb")
                nc.tensor.matmul(pa[:], Wp16[:, ch, 0:128], dfc[:],
                                 start=True, stop=True)
                nc.tensor.matmul(pb[:], Wp16[:, ch, 128:160], dfc[:],
                                 start=True, stop=True)
                if 16 <= ch < 20:
                    k = ch - 16
                    sl = slice(k * 18 * 160, (k + 1) * 18 * 160)
                    nc.sync.dma_start(out=Wpbf[:, sl], in_=dr["Wpb"][:, sl])
                elif ch == 20:
                    nc.sync.dma_start(
                        out=EM64[:].rearrange("p a b -> p (a b)"),
                        in_=dr["EM64"])
                elif ch == 21:
                    for nm, t in [("EXP16", EXP16), ("EXP24", EXP24),
                                  ("MASK1", MASK1), ("MASK24", MASK24),
                                  ("EYE32F", EYE32F)]:
                        nc.sync.dma_start(out=t[:], in_=dr[nm])
                # fused iteration-0 s_j, transposed [b, (c,o)]: xp as the
                # (cheap, 32-row) stationary side, one matmul per chunk
                nc.tensor.matmul(s0p[:], xp16[:, ch, :], Wp16[:, ch, :],
                                 start=(ch == 0), stop=(ch == CH - 1),
                                 skip_group_check=True)
                # drains (psum cols (b, rh') -> 16-contiguous dst runs):
                # U1a splits into halves across DVE and ACT (halves the
                # slot-free latency); batch-quad tail copies split 2:2
                pa_v = pa[:].rearrange("p (b h) -> p b h", b=BC)
                ua_dst = U1a[:, :, 16 * ch:16 * ch + 16]
                nc.vector.tensor_copy(ua_dst[:, 0:16, :], pa_v[:, 0:16, :])
                nc.scalar.copy(ua_dst[:, 16:32, :], pa_v[:, 16:32, :])
                pb_v = pb[:].rearrange("p (b h) -> p b h", b=BC)
                for bq in range(4):
                    dst = U1b4[32 * bq:32 * bq + 32, :, 16 * ch:16 * ch + 16]
                    src = pb_v[:, 8 * bq:8 * bq + 8, :]
                    if bq < 2:
                        nc.vector.tensor_copy(dst, src)
                    else:
                        nc.scalar.copy(dst, src)
            nc.vector.tensor_copy(s0sb[:], s0p[:])

        # routing-phase PSUM pools (opened after the build pools close)
        pp_bb = ctx.enter_context(tc.tile_pool(name="ppbb", bufs=2,
                                               space="PSUM"))
        pp_z = ctx.enter_context(tc.tile_pool(name="ppz", bufs=1, space="PSUM"))
        pp_s = ctx.enter_context(tc.tile_pool(name="pps", bufs=1, space="PSUM"))
        pp_s2 = ctx.enter_context(tc.tile_pool(name="pps2", bufs=1,
                                               space="PSUM"))
        pp_rep = ctx.enter_context(
            tc.tile_pool(name="pprep", bufs=3, space="PSUM"))
        pps = pp_s.tile([128, 512], F32)    # s-diag [16, 0:320] | sq [1,320:352]
        pps2 = pp_s2.tile([128, 512], F32)  # srep [16, 0:320]

        # transpose fused s0 [b, (c,o)] -> pps [16 o, (c, b)]: per c one
        # matmul with s0sb as stationary side against a 32x32 identity
        for c in range(C):
            nc.tensor.matmul(pps[0:16, 32 * c:32 * c + 32],
                             s0sb[:, 16 * c:16 * c + 16], EYE32F[:],
                             start=(c == 0), stop=(c == C - 1),
                             skip_group_check=True)

        def squash(it):
            """pps[0:16, 0:320] holds s-diagonal; emits v + Vm fills / output."""
            stf = bpool.tile([16, 320], F32, tag="stf")
            if it == 0:
                nc.scalar.mul(stf[:], pps[0:16, 0:320], 1.0 / R)
            else:
                nc.vector.tensor_mul(stf[:], pps[0:16, 0:320], rz16[:])
            sq2 = bpool.tile([16, 320], F16, tag="sq2")
            nc.vector.tensor_mul(sq2[:], stf[:], stf[:])
            # sum over o: [16,1]-ones matmul -> [1, 320] at partition base 32
            nc.tensor.matmul(pps2[32:33, 0:320], onesf[:], sq2[:],
                             start=True, stop=True)
            # short chain, all reading sq straight from PSUM:
            #   rt = sqrt(sq+eps); den = (sq+1)*rt; scl = sq/den
            sqp = pps2[32:33, 0:320]
            rt = bpool.tile([1, 320], F32, tag="rt")
            den = bpool.tile([1, 320], F32, tag="den")
            inv = bpool.tile([1, 320], F32, tag="inv")
            scl = bpool.tile([1, 320], F16, tag="scl")
            nc.scalar.activation(rt[:], sqp, SqrtF, bias=epsb[0:1, :])
            if it < 2:
                # prefetch the Exp table for the next iteration's softmax
                nc.scalar.activation(scratch1[:], epsb[0:1, :], ExpF)
            nc.vector.scalar_tensor_tensor(
                den[:], sqp, 1.0, rt[:],
                mybir.AluOpType.add, mybir.AluOpType.mult)
            nc.vector.reciprocal_approx_fast(inv[:], den[:])
            nc.vector.tensor_mul(scl[:], sqp, inv[:])
            nc.tensor.matmul(pps2[0:16, 0:320], ones1[:], scl[:],
                             start=True, stop=True)
            if it < 2:
                vD = bpool.tile([16, 320], F16, tag="vD")
                nc.vector.tensor_mul(vD[:], stf[:], pps2[0:16, 0:320])
                ppv = pp_rep.tile([128, 320], F32, tag="prep")
                nc.tensor.matmul(ppv[0:128, 0:256], EXP16[:], vD[:, 0:256],
                                 start=True, stop=True)
                v24 = vD[:, 256:320].rearrange(
                    "p (c q g) -> p g q c", c=2, g=8)
                nc.tensor.matmul(ppv[0:128, 256:320], EXP24[:], v24,
                                 start=True, stop=True)
                nc.vector.tensor_mul(Vm1[:], ppv[0:128, 0:256], MASK1[:])
                nc.vector.tensor_mul(Vm24[:], ppv[0:128, 256:320], MASK24[:])
            else:
                vD = bpool.tile([16, 320], F32, tag="vDf")
                nc.vector.tensor_mul(vD[:], stf[:], pps2[0:16, 0:320])
                nc.sync.dma_start(out=d_out[:], in_=vD[:])

        squash(0)

        # ================= routing iterations =================
        Vm1v = Vm1[:].rearrange("p (c b) -> p c b", b=BC)       # [128,8,32]
        for it in (1, 2):
            if it > stop_after:
                break
            # software pipeline: b_ij-update block rc on the PE is followed
            # by the rep/y/s_j work of block rc-1, so the PE never drains
            # while DVE (badd/y-mult) and ACT (exp) run under it.
            pz = pp_z.tile([16, 320], F32, tag="pz")

            def bupd_block(rc):
                r0 = 128 * rc
                pbb = pp_bb.tile([128, 384], F32, tag="pbb")
                pbv = pbb[:, 0:320].rearrange("p (c b) -> p c b", b=BC)
                for b in range(BC):
                    nc.tensor.matmul(pbv[:, 0:8, b],
                                     U1a[:, b, r0:r0 + 128],
                                     Vm1v[:, :, b],
                                     start=True, stop=True)
                for bg in range(8):
                    nc.tensor.matmul(
                        pbb[:, 320 + 8 * bg:328 + 8 * bg],
                        U1b4[:, bg, r0:r0 + 128],
                        Vm24[:, 8 * bg:8 * bg + 8],
                        start=True, stop=True)
                # tail cols (bg, bq', cc') -> J-cols 256 + cc'*32 + 8bq' + bg
                h2src = pbb[:, 320:384].rearrange(
                    "p (g q c) -> p g q c", q=4, c=2)
                h2dst = bsb[:, rc, 256:320].rearrange(
                    "p (c q g) -> p g q c", q=4, g=8)
                if it == 1:
                    nc.vector.tensor_copy(bsb[:, rc, 0:256], pbb[:, 0:256])
                    nc.vector.tensor_copy(h2dst, h2src)
                    # exp straight from PSUM, in parallel with the bsb copy
                    nc.scalar.activation(erm[:, rc, 0:256], pbb[:, 0:256],
                                         ExpF)
                    e2dst = erm[:, rc, 256:320].rearrange(
                        "p (c q g) -> p g q c", q=4, g=8)
                    nc.scalar.activation(e2dst, h2src, ExpF)
                else:
                    nc.vector.tensor_add(bsb[:, rc, 0:256], bsb[:, rc, 0:256],
                                         pbb[:, 0:256])
                    nc.vector.tensor_add(h2dst, h2dst, h2src)
                    nc.scalar.activation(erm[:, rc, :], bsb[:, rc, :], ExpF)

            def sj_block(rc):
                nc.tensor.matmul(pz[:], ones16[:], erm[:, rc, :],
                                 start=(rc == 0), stop=(rc == 8),
                                 skip_group_check=True)
                for q in range(8):
                    ch = 8 * rc + q
                    m, par = q // 4, q % 4
                    prep = pp_rep.tile([128, 320], F32, tag="prep")
                    nc.tensor.matmul(prep[:],
                                     EM64[64 * m:64 * m + 64, par, :],
                                     erm[64 * m:64 * m + 64, rc, :],
                                     start=True, stop=True)
                    yc = ystream.tile([128, 320], BF16, tag="yc")
                    xb = xp16[:, ch:ch + 1, :].broadcast_to([128, 10, BC])
                    nc.vector.tensor_mul(
                        yc[:].rearrange("p (c b) -> p c b", b=BC),
                        prep[:].rearrange("p (c b) -> p c b", b=BC), xb)
                    for c in range(C):
                        nc.tensor.matmul(pps[0:16, 32 * c:32 * c + 32],
                                         Wpb[:, ch, 16 * c:16 * c + 16],
                                         yc[:, 32 * c:32 * c + 32],
                                         start=(ch == 0 and c == 0),
                                         stop=(ch == CH - 1 and c == C - 1),
                                         skip_group_check=True)

            for rc in range(9):
                bupd_block(rc)
                if rc >= 1:
                    sj_block(rc - 1)
            # prefetch the Sqrt table for this iteration's squash
            nc.scalar.activation(scratch1[:], epsb[0:1, :], SqrtF)
            sj_block(8)
            nc.vector.reciprocal_approx_fast(rz16[:], pz[:])
            squash(it)


_NC_CACHE = None


def _get_nc():
    global _NC_CACHE
    if _NC_CACHE is None:
        _NC_CACHE = build_nc()
    return _NC_CACHE


def decode_out(o):
    """[16, 320] core output (o, (c, b)) -> [32, 10, 16] (b, c, o)."""
    return o.reshape(O, C, BC).transpose(2, 1, 0)


def kernel(x, W):
    """Full-input entry point. x: [256,1152,8] f32, W: [1152,10,16,8] f32."""
    from concourse.bass_utils import run_bass_kernel_spmd

    x = np.asarray(x, np.float32)
    W = np.asarray(W, np.float32)
    nc = _get_nc()
    consts = _host_consts()
    in_maps = []
    for k in range(NCORES):
        m = _host_prep(x[k * BC:(k + 1) * BC], W)
        m.update(consts)
        in_maps.append(m)
    res = run_bass_kernel_spmd(nc, in_maps, core_ids=list(range(NCORES)))
    v = np.concatenate([decode_out(res.results[k]["out"])
                        for k in range(NCORES)], axis=0)  # [256, 10, 16]
    return v[:, None, :, :, None].astype(np.float32)


# revision 49
# speedup vs baseline: 1.0339x; 1.0155x over previous
"""Trainium2 Bass kernel for nn_CapsuleLayer (dynamic routing capsule layer).

Reference computation (per batch element b):
    u_hat[b,r,c,o] = sum_i W[r,c,o,i] * x[b,r,i]        (R=1152, C=10, O=16, I=8)
    b_ij = 0
    3 routing iterations:
        c_ij = softmax(b_ij, axis=r)
        s_j[c,o] = sum_r c_ij[r,c] * u_hat[r,c,o]
        v = squash(s_j)  over o
        b_ij += sum_o u_hat[r,c,o] * v[c,o]   (except last iteration)
    output v -> [B, 1, C, O, 1]

Sharding: data-parallel over batch B=256 across 8 cores (32 each), W replicated.

Per-core layouts (host numpy prepacks everything; engine SBUF access patterns
must be partition-contiguous and start at partition 0/32/64/96, so every
on-chip tensor here is addressed at partition base 0):
  K-partition index p = 8*rh + i over (rh in [0,16), i in [0,8)); r = 16*ch+rh,
  ch in [0,72).  Column order J = c*32 + b is shared by b_ij, exp, y, the
  s-diagonal PSUM, the squash pipeline and the Vm matrices.

  - u_hat built on PE via block-diagonal "D" matmuls (f16 for precision):
      psum[co, (b, rh')] = sum_p Wp16[p, ch, co] * Df[ch, p, b*16+rh']
    Df columns are (b, rh') so the PSUM->SBUF drains are 16-contiguous:
    U1a[16c+o, b, r] (c<8) via DVE, U1b4[32*bq + 16*cc + o, bg, r]
    (c = 8+cc, b = 8*bq+bg) via the scalar (ACT) engine - both engines
    run the drains in parallel under the PE matmuls.
  - iteration-0 s_j is fused into phase 1: per ch ONE transposed matmul
    (xp16 stationary, 32-row weight load) accumulates s0[b, (c,o)];
    after the loop ten identity-matmuls transpose s0 into the
    [o=16, (c,b)=320] squash layout (replaces the old separate
    720-matmul iteration-0 pass). U1a/U1b4 drains are load-balanced
    across DVE and the scalar engine under the PE matmuls; inputs
    stream in 9-chunk DMA slices with Wpb/consts deferred so the PE
    starts within a few us.
  - b_ij update, r-major directly: per (b, r-chunk-128): lhsT = U1a[:, b, rch]
    (f16 FWL weight loads), rhs = 8-col slice of block-diag(v) Vm1; the tail
    batches 4 b's per matmul via U1b4/Vm24.  All outputs land in one
    [128, 384] J-layout PSUM tile per r-chunk; one DVE add accumulates it
    into b_sb[128, rc, J].
  - softmax (no max subtraction: logits stay small): ACT Exp runs per
    r-block right after each b_ij-update block (for it1 straight from
    PSUM, parallel to the bsb copy) -> erm bf16; Z via ones16
    accumulating matmuls into a [16, 320] PSUM row-block, issued with
    the s_j blocks so they never stall the PE; approx-reciprocal ->
    rz16 [16, 320].
  - s_j diagonal, software-pipelined with the b_ij update (block rc's
    update is followed by block rc-1's rep/y/s_j so the PE never
    drains): per ch: rep = EM64-expand matmul (i-replication of 16 exp
    rows out of a 64-row aligned block); y = rep * xp broadcast over c;
    per c: psum[0:16, 32c:32c+32] += Wpb[:, ch, 16c:16c+16]^T @
    y[:, 32c:+32] -- the c-diagonal lands as column blocks of a single
    [16, 320] region.
  - squash entirely in the [o=16, (c, b)] layout: s~ = s * rz; sum_o via a
    [16,1]-ones matmul -> [1, 320]; scale chain on [1, 320]; scale
    replicated back to 16 rows by a K=1 ones matmul; v = s~ * scale.
    Dummy 1-element ACT ops prefetch the Exp/Sqrt tables during engine-idle
    windows so the table swaps stay off the critical path.
  - Vm fill without partition gymnastics: vFull = EXP-matmul replicating vD's
    o-rows to all 128 capsule rows, then Vm = vFull * MASK (host constants
    carry the c-diagonal / batch-quad block structure).
  - output vD [o, (c, b)] fp32 -> DRAM [16, 320]; numpy -> [B,1,C,O,1].
"""

import sys

if "/opt/trn_rl_repo" not in sys.path:
    sys.path.insert(0, "/opt/trn_rl_repo")

import numpy as np
import ml_dtypes

import concourse.bass as bass
import concourse.mybir as mybir
from concourse import bacc
from concourse.tile import TileContext

BF16 = mybir.dt.bfloat16
F16 = mybir.dt.float16
F32 = mybir.dt.float32
NPBF16 = ml_dtypes.bfloat16

B, R, C, O, I = 256, 1152, 10, 16, 8
NCORES = 8
BC = B // NCORES          # 32 batch elements per core
CH = R // 16              # 72 chunks of 16 r's
EPS = 1e-7
ExpF = mybir.ActivationFunctionType.Exp
SqrtF = mybir.ActivationFunctionType.Sqrt


def _host_prep(xs, W):
    """Per-core input arrays. xs: [32,1152,8] f32, W: [1152,10,16,8] f32."""
    Wr = (
        W.reshape(CH, 16, C, O, I)
        .transpose(1, 4, 0, 2, 3)  # rh, i, ch, c, o
        .reshape(128, CH * 160)
    )
    Wp16 = Wr.astype(np.float16)
    Wpb = Wr.astype(NPBF16)
    xr = (
        xs.reshape(BC, CH, 16, I)
        .transpose(2, 3, 1, 0)  # rh, i, ch, b
        .reshape(128, CH * BC)
    )
    xp16 = xr.astype(np.float16)
    tmp = xs.reshape(BC, CH, 16, I).transpose(1, 2, 3, 0)  # ch, rh, i, b
    # D block-diag with (b, rh') column order -> 16-contiguous drains;
    # chunk PAIRS are packed side by side per partition so one DMA (with
    # 2KB descriptor runs) delivers two chunks
    D6 = np.zeros((CH, 16, I, BC, 16), np.float32)
    for rh in range(16):
        D6[:, rh, :, :, rh] = tmp[:, rh, :, :]
    Df = (D6.reshape(CH // 2, 2, 128, BC * 16)
          .transpose(0, 2, 1, 3)
          .reshape(CH // 2, 128, 2 * BC * 16)
          .astype(np.float16))
    return {"Wp16": Wp16, "Wpb": Wpb, "xp16": xp16, "Df": Df}


def _host_consts():
    p = np.arange(128)
    # EM64[64k + 16*par + j2, par*128 + q] = (q//8 == j2): K=64 expand blocks
    # at partition bases 0/64; col group `par` selects which 16-row quarter of
    # the 64-block is replicated into the (rh, i) partition grid.
    EM64 = np.zeros((128, 4, 128), np.float32)
    for k in range(2):
        for par in range(4):
            for j2 in range(16):
                EM64[64 * k + 16 * par + j2, par, :] = (p // 8 == j2)
    EM64 = EM64.reshape(128, 512).astype(NPBF16)
    # EXP16[o', 16c+o] = (o'==o): replicate vD rows to all capsule rows
    EXP16 = np.tile(np.eye(16, dtype=np.float32), (1, 8)).astype(np.float16)
    # EXP24[o', 32bq+16cc+o] = (o'==o)
    EXP24 = np.tile(np.tile(np.eye(16, dtype=np.float32), (1, 2)),
                    (1, 4)).astype(np.float16)
    # MASK1[16c+o, c'*32+b] = (c==c') for c' < 8
    c_of = (p // 16)[:, None]
    cols = np.arange(256)[None, :] // 32
    MASK1 = (c_of == cols).astype(np.float16)
    # MASK24[32bq+16cc+o, bg*8+bq'*2+cc'] = (bq==bq') & (cc==cc')
    bq_of = (p // 32)[:, None]
    cc_of = ((p % 32) // 16)[:, None]
    col24 = np.arange(64)[None, :]
    MASK24 = ((bq_of == (col24 % 8) // 2) & (cc_of == col24 % 2)).astype(
        np.float16)
    EYE32F = np.eye(32, dtype=np.float32)
    return {"EM64": EM64, "EXP16": EXP16, "EXP24": EXP24, "MASK1": MASK1,
            "MASK24": MASK24, "EYE32F": EYE32F}


def build_nc(stop_after=2):
    nc = bacc.Bacc("TRN2", target_bir_lowering=False, debug=False,
                   num_devices=NCORES)
    dr = {}
    for name, shape, dt in [
        ("Wp16", [128, CH * 160], F16), ("Wpb", [128, CH * 160], BF16),
        ("xp16", [128, CH * BC], F16),
        ("Df", [CH // 2, 128, 32 * BC], F16), ("EM64", [128, 512], BF16),
        ("EXP16", [16, 128], F16), ("EXP24", [16, 128], F16),
        ("MASK1", [128, 256], F16), ("MASK24", [128, 64], F16),
        ("EYE32F", [32, 32], F32),
    ]:
        dr[name] = nc.dram_tensor(name, shape, dt, kind="ExternalInput").ap()
    d_out = nc.dram_tensor("out", [O, 320], F32, kind="ExternalOutput").ap()

    with TileContext(nc) as tc:
        _emit(nc, tc, dr, d_out, stop_after)
    nc.compile()
    return nc


def _emit(nc, tc, dr, d_out, stop_after=2):
    from contextlib import ExitStack

    with ExitStack() as ctx:
        consts = ctx.enter_context(tc.tile_pool(name="consts", bufs=1))
        upool = ctx.enter_context(tc.tile_pool(name="upool", bufs=1))
        bpool = ctx.enter_context(tc.tile_pool(name="bpool", bufs=1))
        dstream = ctx.enter_context(tc.tile_pool(name="dstream", bufs=6))
        ystream = ctx.enter_context(tc.tile_pool(name="ystream", bufs=4))

        # ---- resident tiles ----
        Wp16 = consts.tile([128, CH, 160], F16)
        Wpb = consts.tile([128, CH, 160], BF16)
        xp16 = consts.tile([128, CH, BC], F16)
        EM64 = consts.tile([128, 4, 128], BF16)
        EXP16 = consts.tile([16, 128], F16)
        EXP24 = consts.tile([16, 128], F16)
        MASK1 = consts.tile([128, 256], F16)
        MASK24 = consts.tile([128, 64], F16)
        EYE32F = consts.tile([32, 32], F32)
        ones16 = consts.tile([128, 16], BF16)
        onesf = consts.tile([16, 1], F16)
        ones1 = consts.tile([1, 16], F16)
        epsb = consts.tile([128, 1], F32)
        scratch1 = consts.tile([1, 1], F32)

        # issue order tracks the critical path: the very first compute
        # (s0T/pa of chunk 0) needs only xp slice 0, Wp16 slice 0 and Df[0],
        # so those three dma_starts go first (~610ns SP issue cost each);
        # the remaining slices and Df prefetches follow
        xp16f = xp16[:].rearrange("p a b -> p (a b)")
        Wp16f = Wp16[:].rearrange("p a b -> p (a b)")
        Wpbf = Wpb[:].rearrange("p a b -> p (a b)")
        nc.sync.dma_start(out=xp16f[:, 0:9 * BC], in_=dr["xp16"][:, 0:9 * BC])
        nc.sync.dma_start(out=Wp16f[:, 0:9 * 160], in_=dr["Wp16"][:, 0:9 * 160])
        early_dfc = []
        for j in range(2):
            t = dstream.tile([128, 2, 16 * BC], F16, tag="dfc")
            nc.sync.dma_start(out=t[:].rearrange("p a b -> p (a b)"),
                              in_=dr["Df"][j])
            early_dfc.append(t)
        for k in range(1, 8):
            slx = slice(k * 9 * BC, (k + 1) * 9 * BC)
            nc.sync.dma_start(out=xp16f[:, slx], in_=dr["xp16"][:, slx])
            sl = slice(k * 9 * 160, (k + 1) * 9 * 160)
            nc.sync.dma_start(out=Wp16f[:, sl], in_=dr["Wp16"][:, sl])
        nc.gpsimd.memset(ones16[:], 1.0)
        nc.gpsimd.memset(onesf[:], 1.0)
        nc.gpsimd.memset(ones1[:], 1.0)
        nc.gpsimd.memset(epsb[:], EPS)

        U1a = upool.tile([128, BC, R], F16)     # [16c+o (c<8), b, r]
        U1b4 = upool.tile([128, 8, R], F16)     # [32bq+16cc+o, bg, r]

        bsb = bpool.tile([128, 9, 320], F32)    # b_ij r-major, cols J=c*32+b
        erm = bpool.tile([128, 9, 320], BF16)   # exp(b_ij)
        rz16 = bpool.tile([16, 320], F32)       # 1/Z on 16 o-rows
        Vm1 = bpool.tile([128, 256], F16)       # block-diag v, cols c*32+b
        Vm24 = bpool.tile([128, 64], F16)       # [32bq+16cc+o, (bq',cc',bg)]
        s0sb = bpool.tile([32, 160], F32)       # fused it0 s_j, [b, (c,o)]

        # preload the Sqrt table while phase 1 runs (squash(0) needs it)
        nc.scalar.activation(scratch1[:], epsb[0:1, :], SqrtF)

        # ===== Phase 1: build u_hat + fused iteration-0 s_j =====
        with tc.tile_pool(name="ppbuild", bufs=4, space="PSUM") as ppb1, \
             tc.tile_pool(name="ppbuild2", bufs=3, space="PSUM") as ppb2, \
             tc.tile_pool(name="pps0", bufs=1, space="PSUM") as pps0:
            s0p = pps0.tile([32, 160], F32)
            dfc2 = None
            for ch in range(CH):
                j = ch // 2
                if ch % 2 == 0:
                    if j < 2:
                        dfc2 = early_dfc[j]
                    else:
                        dfc2 = dstream.tile([128, 2, 16 * BC], F16,
                                            tag="dfc")
                        nc.sync.dma_start(
                            out=dfc2[:].rearrange("p a b -> p (a b)"),
                            in_=dr["Df"][j])
                dfc = dfc2[:, ch % 2, :]
                pa = ppb1.tile([128, 16 * BC], F32, tag="pa")
                pb = ppb2.tile([32, 16 * BC], F32, tag="pb")
                nc.tensor.matmul(pa[:], Wp16[:, ch, 0:128], dfc[:],
                                 start=True, stop=True)
                nc.tensor.matmul(pb[:], Wp16[:, ch, 128:160], dfc[:],
                                 start=True, stop=True)
                if 16 <= ch < 20:
                    k = ch - 16
                    sl = slice(k * 18 * 160, (k + 1) * 18 * 160)
                    nc.sync.dma_start(out=Wpbf[:, sl], in_=dr["Wpb"][:, sl])
                elif ch == 20:
                    nc.sync.dma_start(
                        out=EM64[:].rearrange("p a b -> p (a b)"),
                        in_=dr["EM64"])
                elif ch == 21:
                    for nm, t in [("EXP16", EXP16), ("EXP24", EXP24),
                                  ("MASK1", MASK1), ("MASK24", MASK24),
                                  ("EYE32F", EYE32F)]:
                        nc.sync.dma_start(out=t[:], in_=dr[nm])
                # fused iteration-0 s_j, transposed [b, (c,o)]: xp as the
                # (cheap, 32-row) stationary side, one matmul per chunk
                nc.tensor.matmul(s0p[:], xp16[:, ch, :], Wp16[:, ch, :],
                                 start=(ch == 0), stop=(ch == CH - 1),
                                 skip_group_check=True)
                # drains (psum cols (b, rh') -> 16-contiguous dst runs):
                # U1a splits into halves across DVE and ACT (halves the
                # slot-free latency); batch-quad tail copies split 2:2
                pa_v = pa[:].rearrange("p (b h) -> p b h", b=BC)
                ua_dst = U1a[:, :, 16 * ch:16 * ch + 16]
                nc.vector.tensor_copy(ua_dst[:, 0:16, :], pa_v[:, 0:16, :])
                nc.scalar.copy(ua_dst[:, 16:32, :], pa_v[:, 16:32, :])
                pb_v = pb[:].rearrange("p (b h) -> p b h", b=BC)
                for bq in range(4):
                    dst = U1b4[32 * bq:32 * bq + 32, :, 16 * ch:16 * ch + 16]
                    src = pb_v[:, 8 * bq:8 * bq + 8, :]
                    if bq < 2:
                        nc.vector.tensor_copy(dst, src)
                    else:
                        nc.scalar.copy(dst, src)
            nc.vector.tensor_copy(s0sb[:], s0p[:])

        # routing-phase PSUM pools (opened after the build pools close)
        pp_bb = ctx.enter_context(tc.tile_pool(name="ppbb", bufs=3,
                                               space="PSUM"))
        pp_z = ctx.enter_context(tc.tile_pool(name="ppz", bufs=1, space="PSUM"))
        pp_s = ctx.enter_context(tc.tile_pool(name="pps", bufs=1, space="PSUM"))
        pp_s2 = ctx.enter_context(tc.tile_pool(name="pps2", bufs=1,
                                               space="PSUM"))
        pp_rep = ctx.enter_context(
            tc.tile_pool(name="pprep", bufs=2, space="PSUM"))
        pps = pp_s.tile([128, 512], F32)    # s-diag [16, 0:320] | sq [1,320:352]
        pps2 = pp_s2.tile([128, 512], F32)  # srep [16, 0:320]

        # transpose fused s0 [b, (c,o)] -> pps [16 o, (c, b)]: per c one
        # matmul with s0sb as stationary side against a 32x32 identity
        for c in range(C):
            nc.tensor.matmul(pps[0:16, 32 * c:32 * c + 32],
                             s0sb[:, 16 * c:16 * c + 16], EYE32F[:],
                             start=(c == 0), stop=(c == C - 1),
                             skip_group_check=True)

        def squash(it):
            """pps[0:16, 0:320] holds s-diagonal; emits v + Vm fills / output."""
            stf = bpool.tile([16, 320], F32, tag="stf")
            if it == 0:
                nc.scalar.mul(stf[:], pps[0:16, 0:320], 1.0 / R)
            else:
                nc.vector.tensor_mul(stf[:], pps[0:16, 0:320], rz16[:])
            sq2 = bpool.tile([16, 320], F16, tag="sq2")
            nc.vector.tensor_mul(sq2[:], stf[:], stf[:])
            # sum over o: [16,1]-ones matmul -> [1, 320] at partition base 32
            nc.tensor.matmul(pps2[32:33, 0:320], onesf[:], sq2[:],
                             start=True, stop=True)
            # short chain, all reading sq straight from PSUM:
            #   rt = sqrt(sq+eps); den = (sq+1)*rt; scl = sq/den
            sqp = pps2[32:33, 0:320]
            rt = bpool.tile([1, 320], F32, tag="rt")
            den = bpool.tile([1, 320], F32, tag="den")
            inv = bpool.tile([1, 320], F32, tag="inv")
            scl = bpool.tile([1, 320], F16, tag="scl")
            nc.scalar.activation(rt[:], sqp, SqrtF, bias=epsb[0:1, :])
            if it < 2:
                # prefetch the Exp table for the next iteration's softmax
                nc.scalar.activation(scratch1[:], epsb[0:1, :], ExpF)
            nc.vector.scalar_tensor_tensor(
                den[:], sqp, 1.0, rt[:],
                mybir.AluOpType.add, mybir.AluOpType.mult)
            nc.vector.reciprocal_approx_fast(inv[:], den[:])
            nc.vector.tensor_mul(scl[:], sqp, inv[:])
            nc.tensor.matmul(pps2[0:16, 0:320], ones1[:], scl[:],
                             start=True, stop=True)
            if it < 2:
                vD = bpool.tile([16, 320], F16, tag="vD")
                nc.vector.tensor_mul(vD[:], stf[:], pps2[0:16, 0:320])
                ppv = pps2
                nc.tensor.matmul(ppv[0:128, 0:256], EXP16[:], vD[:, 0:256],
                                 start=True, stop=True)
                v24 = vD[:, 256:320].rearrange(
                    "p (c q g) -> p g q c", c=2, g=8)
                nc.tensor.matmul(ppv[0:128, 256:320], EXP24[:], v24,
                                 start=True, stop=True)
                nc.vector.tensor_mul(Vm1[:], ppv[0:128, 0:256], MASK1[:])
                nc.vector.tensor_mul(Vm24[:], ppv[0:128, 256:320], MASK24[:])
            else:
                vD = bpool.tile([16, 320], F32, tag="vDf")
                nc.vector.tensor_mul(vD[:], stf[:], pps2[0:16, 0:320])
                nc.sync.dma_start(out=d_out[:], in_=vD[:])

        squash(0)

        # ================= routing iterations =================
        Vm1v = Vm1[:].rearrange("p (c b) -> p c b", b=BC)       # [128,8,32]
        for it in (1, 2):
            if it > stop_after:
                break
            # software pipeline: b_ij-update block rc on the PE is followed
            # by the rep/y/s_j work of block rc-1, so the PE never drains
            # while DVE (badd/y-mult) and ACT (exp) run under it.
            pz = pp_z.tile([16, 320], F32, tag="pz")

            def bupd_block(rc):
                r0 = 128 * rc
                pbb = pp_bb.tile([128, 384], F32, tag="pbb")
                pbv = pbb[:, 0:320].rearrange("p (c b) -> p c b", b=BC)
                for b in range(BC):
                    nc.tensor.matmul(pbv[:, 0:8, b],
                                     U1a[:, b, r0:r0 + 128],
                                     Vm1v[:, :, b],
                                     start=True, stop=True)
                for bg in range(8):
                    nc.tensor.matmul(
                        pbb[:, 320 + 8 * bg:328 + 8 * bg],
                        U1b4[:, bg, r0:r0 + 128],
                        Vm24[:, 8 * bg:8 * bg + 8],
                        start=True, stop=True)
                # tail cols (bg, bq', cc') -> J-cols 256 + cc'*32 + 8bq' + bg
                h2src = pbb[:, 320:384].rearrange(
                    "p (g q c) -> p g q c", q=4, c=2)
                h2dst = bsb[:, rc, 256:320].rearrange(
                    "p (c q g) -> p g q c", q=4, g=8)
                if it == 1:
                    nc.vector.tensor_copy(bsb[:, rc, 0:256], pbb[:, 0:256])
                    nc.vector.tensor_copy(h2dst, h2src)
                    # exp straight from PSUM, in parallel with the bsb copy
                    nc.scalar.activation(erm[:, rc, 0:256], pbb[:, 0:256],
                                         ExpF)
                    e2dst = erm[:, rc, 256:320].rearrange(
                        "p (c q g) -> p g q c", q=4, g=8)
                    nc.scalar.activation(e2dst, h2src, ExpF)
                else:
                    nc.vector.tensor_add(bsb[:, rc, 0:256], bsb[:, rc, 0:256],
                                         pbb[:, 0:256])
                    nc.vector.tensor_add(h2dst, h2dst, h2src)
                    nc.scalar.activation(erm[:, rc, :], bsb[:, rc, :], ExpF)

            def sj_block(rc):
                nc.tensor.matmul(pz[:], ones16[:], erm[:, rc, :],
                                 start=(rc == 0), stop=(rc == 8),
                                 skip_group_check=True)
                for q in range(8):
                    ch = 8 * rc + q
                    m, par = q // 4, q % 4
                    prep = pp_rep.tile([128, 320], F32, tag="prep")
                    nc.tensor.matmul(prep[:],
                                     EM64[64 * m:64 * m + 64, par, :],
                                     erm[64 * m:64 * m + 64, rc, :],
                                     start=True, stop=True)
                    yc = ystream.tile([128, 320], BF16, tag="yc")
                    xb = xp16[:, ch:ch + 1, :].broadcast_to([128, 10, BC])
                    nc.vector.tensor_mul(
                        yc[:].rearrange("p (c b) -> p c b", b=BC),
                        prep[:].rearrange("p (c b) -> p c b", b=BC), xb)
                    for c in range(C):
                        nc.tensor.matmul(pps[0:16, 32 * c:32 * c + 32],
                                         Wpb[:, ch, 16 * c:16 * c + 16],
                                         yc[:, 32 * c:32 * c + 32],
                                         start=(ch == 0 and c == 0),
                                         stop=(ch == CH - 1 and c == C - 1),
                                         skip_group_check=True)

            # lag-2 pipeline: two b_ij-update blocks run ahead of the
            # s_j work so the PE has ready matmuls while the first
            # exp/softmax chain is still in flight on DVE/ACT
            for rc in range(9):
                bupd_block(rc)
                if rc >= 2:
                    sj_block(rc - 2)
            # prefetch the Sqrt table for this iteration's squash
            nc.scalar.activation(scratch1[:], epsb[0:1, :], SqrtF)
            sj_block(7)
            sj_block(8)
            nc.vector.reciprocal_approx_fast(rz16[:], pz[:])
            squash(it)


_NC_CACHE = None


def _get_nc():
    global _NC_CACHE
    if _NC_CACHE is None:
        _NC_CACHE = build_nc()
    return _NC_CACHE


def decode_out(o):
    """[16, 320] core output (o, (c, b)) -> [32, 10, 16] (b, c, o)."""
    return o.reshape(O, C, BC).transpose(2, 1, 0)


def kernel(x, W):
    """Full-input entry point. x: [256,1152,8] f32, W: [1152,10,16,8] f32."""
    from concourse.bass_utils import run_bass_kernel_spmd

    x = np.asarray(x, np.float32)
    W = np.asarray(W, np.float32)
    nc = _get_nc()
    consts = _host_consts()
    in_maps = []
    for k in range(NCORES):
        m = _host_prep(x[k * BC:(k + 1) * BC], W)
        m.update(consts)
        in_maps.append(m)
    res = run_bass_kernel_spmd(nc, in_maps, core_ids=list(range(NCORES)))
    v = np.concatenate([decode_out(res.results[k]["out"])
                        for k in range(NCORES)], axis=0)  # [256, 10, 16]
    return v[:, None, :, :, None].astype(np.float32)
